# revision 82
# baseline (speedup 1.0000x reference)
"""Masked attention (B=16, QT=KT=2048, D=1024, fp32) on 8 Trainium2 NeuronCores.

Strategy:
 - Work unit = (128 q rows) x (512 k cols) partial attention with online
   (flash-style) softmax accumulation; k-outer / q-inner inside a "fragment".
 - A fragment = (NQ q-tiles) x (NK k-chunks) of one batch; every core runs an
   identical static sequence of fragment shapes (SPMD), host packs which
   (batch, q-range) goes where, padding with dummy slots.
 - Per-batch length specialization: only k-chunks < ceil(K_len/512) and
   q-tiles < ceil(Q_len/128) are computed. Invalid q rows are fixed up on the
   host (reference semantics: fully-masked rows -> uniform average of V).
 - Numerics (default "fp16" mode): Q/K/V/A in fp16 (11-bit significand),
   S accumulated in fp32 PSUM, softmax fully in fp32 on ScalarE/VectorE,
   additive length-mask folded into the S matmul group as a rank-1
   (ones x mask_row) matmul. End-to-end absmax relative error vs the fp32
   reference ~9e-3 (gate is 2e-2). "bf16x3" mode (3-pass hi/lo split QK)
   gives ~4e-4 at ~1.4x the time.
"""

import os
import numpy as np
import ml_dtypes
from contextlib import ExitStack

import concourse.bass as bass
import concourse.tile as tile
from concourse import bacc, mybir
from concourse.bass_utils import run_bass_kernel_spmd

F32 = mybir.dt.float32
F32R = mybir.dt.float32r
BF16 = mybir.dt.bfloat16
FP16 = mybir.dt.float16
AF = mybir.ActivationFunctionType
ALU = mybir.AluOpType

B, QT, KT, D = 16, 2048, 2048, 1024
QTILE, KCH = 128, 512
NCORES = 8
DCH = D // 128          # 8 contraction chunks of 128
KBLK = KCH // 128       # 4 k sub-blocks per chunk (transpose granularity)
MASK_NEG = float(-(2 ** 32))
NQ_MAX = 6

_PROG_CACHE: dict = {}
LAST_EXEC_NS = [None]

# QK_MODE:
#  "fp16"   - full fp16 pipeline (Q,K,V,A fp16; 1-pass QK). ~1e-2 relerr,
#             fastest (half the DMA bytes of tf32, fp16 transposes).
#  "tf32"   - 1-pass float32r QK, tf32 V/A. ~9e-3 relerr.
#  "bf16x3" - 3-pass bf16 hi/lo QK, tf32 V/A. ~4e-4 relerr, ~1.5x slower.
QK_MODE = os.environ.get("ATTN_QK_MODE", "fp16")

# DMA_R per mode: per-chunk DMA time / per-unit PE time (for the planner)
_DMA_R = {"bf16x3": 2.2, "tf32": 3.8, "fp16": float(os.environ.get("ATTN_DMA_R", "1.0"))}


def _tf32(x):
    b = np.ascontiguousarray(x, dtype=np.float32).view(np.uint32)
    rb = (b >> 13) & np.uint32(1)
    b = (b + np.uint32(0x0FFF) + rb) & np.uint32(0xFFFFE000)
    return b.view(np.float32)


# --------------------------------------------------------------------------
# planning: choose fragment shape classes + assign (batch, q-run) fragments
# --------------------------------------------------------------------------

def _slot_lists(max_nq, max_len=3, max_sum=14):
    """All non-increasing slot-capacity lists."""
    out = []

    def rec(prefix, lo):
        if prefix:
            out.append(tuple(prefix))
        if len(prefix) >= max_len:
            return
        for s in range(min(max_nq, lo), 0, -1):
            if sum(prefix) + s <= max_sum:
                rec(prefix + [s], s)

    rec([], max_nq)
    return out


def _try_pack(sizes, slots):
    """Greedy: place batches (id, q-tiles) into slots (each has NCORES
    positions, one single-batch run per position, run_len <= capacity).
    Returns list of (batch, q0, ln, slot_idx) or None."""
    rem = {b: n for b, n in sizes.items() if n > 0}
    offs = {b: 0 for b in rem}
    avail = []  # (capacity, slot_idx) positions
    for si, cap in enumerate(slots):
        avail += [(cap, si)] * NCORES
    avail.sort(reverse=True)
    placement = []
    while rem:
        b = max(rem, key=lambda b: rem[b])
        if not avail:
            return None
        cap, si = avail.pop(0)
        ln = min(rem[b], cap)
        placement.append((b, offs[b], ln, si))
        offs[b] += ln
        rem[b] -= ln
        if rem[b] == 0:
            del rem[b]
    return placement


def _plan(nqt, nkt, DMA_R=2.5):
    """Returns (groups, assign):
    groups: list of (NK, NQ, F=1) executed by every core in order.
    assign: dict (core, group_idx, 0) -> (batch, q_tile_start, run_len)
    DMA_R: per-chunk DMA time over per-unit PE time; a fragment costs
    max(compute, chunk-DMA) per k-chunk.
    """
    batches = list(range(len(nqt)))
    distinct = sorted({nkt[b] for b in batches if nqt[b] > 0})
    if not distinct:
        return [], {}
    cand_slots = _slot_lists(NQ_MAX) + _slot_lists(NQ_MAX, max_len=6, max_sum=40)

    def best_config(nks):
        bs = {b: nqt[b] for b in batches if nkt[b] in nks and nqt[b] > 0}
        if not bs:
            return (0.0, (), None)
        NK = max(nkt[b] for b in bs)
        best = None
        for slots in cand_slots:
            if NCORES * sum(slots) < sum(bs.values()):
                continue
            pl = _try_pack(bs, slots)
            if pl is None:
                continue
            cost = NK * sum(max(s, DMA_R) for s in slots)
            key = (cost, sum(slots), len(slots))
            if best is None or key < best[3]:
                best = (cost, slots, pl, key)
        if best is None:
            return (float("inf"), (), None)
        return best[:3]

    best_total, best_parts = None, None
    n = len(distinct)
    for mask in range(1 << max(0, n - 1)):
        parts, start = [], 0
        for i in range(n - 1):
            if mask >> i & 1:
                parts.append(distinct[start:i + 1])
                start = i + 1
        parts.append(distinct[start:])
        total = sum(best_config(p)[0] for p in parts)
        if best_total is None or total < best_total:
            best_total, best_parts = total, parts
    assert best_parts is not None and best_total != float("inf"), "no feasible plan"

    groups, assign = [], {}
    for part in best_parts:
        bs = [b for b in batches if nkt[b] in part and nqt[b] > 0]
        if not bs:
            continue
        NK = max(nkt[b] for b in bs)
        _, slots, placement = best_config(part)
        gi0 = len(groups)
        for cap in slots:
            groups.append((NK, cap, 1))
        used = {}  # slot_idx -> next core
        for (b, q0, ln, si) in placement:
            c = used.get(si, 0)
            used[si] = c + 1
            assign[(c, gi0 + si, 0)] = (b, q0, ln)
    # large-NK groups first: compute-dense start keeps PE warm; small
    # fragments at the tail ride on already-prefetched DMA
    order = sorted(range(len(groups)), key=lambda i: (-groups[i][0], -groups[i][1]))
    remap = {old: new for new, old in enumerate(order)}
    groups = [groups[i] for i in order]
    assign = {(c, remap[g], f): v for (c, g, f), v in assign.items()}
    return groups, assign


# --------------------------------------------------------------------------
# device program (cached by fragment-shape signature)
# --------------------------------------------------------------------------

def _build_program(groups, qk_mode):
    TQ = sum(NQ * F for (_, NQ, F) in groups)
    CH = sum(NK * F for (NK, _, F) in groups)
    multi_pass = qk_mode == "bf16x3"
    if qk_mode == "fp16":
        QDT = FP16        # Q/K/mask dtype
        AVDT = FP16       # A (post-exp) and V dtype
        TDT = FP16        # transpose data dtype
    elif qk_mode == "tf32":
        QDT, AVDT, TDT = F32R, F32R, F32
    else:
        QDT, AVDT, TDT = BF16, F32R, F32

    nc = bacc.Bacc("TRN2", target_bir_lowering=False, debug=False)
    qh_e = nc.dram_tensor("qh", [TQ, DCH, 128, QTILE], QDT, kind="ExternalInput")
    kh_e = nc.dram_tensor("kh", [CH, DCH, 128, KCH], QDT, kind="ExternalInput")
    if multi_pass:
        ql_e = nc.dram_tensor("ql", [TQ, DCH, 128, QTILE], BF16, kind="ExternalInput")
        kl_e = nc.dram_tensor("kl", [CH, DCH, 128, KCH], BF16, kind="ExternalInput")
    v_e = nc.dram_tensor("v", [CH, KBLK, 128, D], AVDT, kind="ExternalInput")
    mk_e = nc.dram_tensor("mask", [CH, 128, KCH], QDT, kind="ExternalInput")
    id_e = nc.dram_tensor("ident", [128, 128], TDT, kind="ExternalInput")
    o_e = nc.dram_tensor("o", [TQ, 128, D], F32, kind="ExternalOutput")

    with tile.TileContext(nc) as tc:
        with ExitStack() as ctx:
            const = ctx.enter_context(tc.tile_pool(name="const", bufs=1))
            deep = 7 if qk_mode == "fp16" else 3
            qpool = ctx.enter_context(tc.tile_pool(name="qpool", bufs=2))
            kpool = ctx.enter_context(tc.tile_pool(name="kpool", bufs=deep))
            vpool = ctx.enter_context(tc.tile_pool(name="vpool", bufs=deep))
            mpool = ctx.enter_context(tc.tile_pool(name="mpool", bufs=deep))
            state = ctx.enter_context(tc.tile_pool(name="state", bufs=2))
            work = ctx.enter_context(tc.tile_pool(name="work", bufs=3))
            small = ctx.enter_context(tc.tile_pool(name="small", bufs=6))
            opool = ctx.enter_context(tc.tile_pool(name="opool", bufs=2))
            ps_s = ctx.enter_context(tc.tile_pool(name="ps_s", bufs=2, space="PSUM"))
            ps_t = ctx.enter_context(tc.tile_pool(name="ps_t", bufs=2, space="PSUM"))
            ps_o = ctx.enter_context(tc.tile_pool(name="ps_o", bufs=2, space="PSUM"))

            ident = const.tile([128, 128], TDT)
            nc.sync.dma_start(ident[:], id_e[:])
            # HAM warm-up: ~4us of dummy matmuls during the initial DMA ramp
            # so the PE clock is at 8/8 when real work starts
            for w in range(40):
                wp = ps_t.tile([128, 128], F32, tag="ptp")
                nc.tensor.matmul(wp[:], ident[:], ident[:], start=True,
                                 stop=True)

            qslot = 0
            chslot = 0
            for (NK, NQ, F) in groups:
                for f in range(F):
                    # fragment state (not needed for single-chunk fragments)
                    if NK > 1:
                        mbar = state.tile([128, NQ], F32, tag="mbar")
                        dst = state.tile([128, NQ], F32, tag="dst")
                        oacc = state.tile([128, NQ * D], F32, tag="oacc")

                    # load this fragment's q tiles
                    qh = qpool.tile([128, NQ, DCH, QTILE], QDT, tag="qh")
                    if multi_pass:
                        ql = qpool.tile([128, NQ, DCH, QTILE], BF16, tag="ql")
                    for t in range(NQ):
                        nc.gpsimd.dma_start(
                            qh[:, t], qh_e[qslot + t].rearrange("c p q -> p c q"))
                        if multi_pass:
                            nc.gpsimd.dma_start(
                                ql[:, t], ql_e[qslot + t].rearrange("c p q -> p c q"))

                    for j in range(NK):
                        kh = kpool.tile([128, DCH, KCH], QDT, tag="kh")
                        vv = vpool.tile([128, KBLK, D], AVDT, tag="v")
                        mk = mpool.tile([128, KCH], QDT, tag="mk")
                        nc.sync.dma_start(
                            kh[:], kh_e[chslot + j].rearrange("c p k -> p c k"))
                        if multi_pass:
                            kl = kpool.tile([128, DCH, KCH], BF16, tag="kl")
                            nc.sync.dma_start(kl[:], kl_e[chslot + j].rearrange("c p k -> p c k"))
                        nc.sync.dma_start(vv[:], v_e[chslot + j].rearrange("c p d -> p c d"))
                        nc.sync.dma_start(mk[:], mk_e[chslot + j])

                        for t in range(NQ):
                            # S accumulation in fp32 PSUM
                            sp = ps_s.tile([128, KCH], F32, tag="sp")
                            n_mm = (3 if multi_pass else 1) * DCH
                            i = 0
                            for c in range(DCH):
                                khc = kh[:, c]
                                if multi_pass:
                                    passes = [(qh[:, t, c], khc),
                                              (qh[:, t, c], kl[:, c]),
                                              (ql[:, t, c], khc)]
                                else:
                                    passes = [(qh[:, t, c], khc)]
                                for (lhs, rhs) in passes:
                                    nc.tensor.matmul(
                                        sp[:], lhs, rhs,
                                        start=(i == 0), stop=(i == n_mm - 1))
                                    i += 1

                            # additive length mask on DVE (frees PE cycles,
                            # releases the S PSUM bank early)
                            s_sb = work.tile([128, KCH], F32, tag="s_sb")
                            nc.vector.tensor_add(s_sb[:], sp[:], mk[:])
                            mbj = small.tile([128, 1], F32, tag="mbj")
                            nc.vector.tensor_reduce(
                                mbj[:], s_sb[:], axis=mybir.AxisListType.X,
                                op=ALU.max, negate=True)

                            st = slice(t, t + 1)
                            if j == 0:
                                if NK > 1:
                                    nc.vector.tensor_copy(mbar[:, st], mbj[:])
                                mnew = mbj
                            else:
                                mnew = small.tile([128, 1], F32, tag="mnew")
                                nc.vector.tensor_tensor(
                                    mnew[:], mbj[:], mbar[:, st], ALU.min)
                                alpha = small.tile([128, 1], F32, tag="alpha")
                                # alpha = exp(m_old - m_new) = exp(mnew_bar - mold_bar)
                                nc.scalar.activation(
                                    alpha[:], mbar[:, st], AF.Exp,
                                    bias=mnew[:], scale=-1.0)
                                if j < NK - 1:
                                    nc.vector.tensor_copy(mbar[:, st], mnew[:])

                            # P = exp(S - m), row sums (reads PSUM directly)
                            p_sb = work.tile([128, KCH], TDT, tag="p_sb")
                            sj = small.tile([128, 1], F32, tag="sj")
                            nc.scalar.activation(
                                p_sb[:], s_sb[:], AF.Exp, bias=mnew[:], scale=1.0,
                                accum_out=sj[:])

                            if NK > 1:
                                if j == 0:
                                    nc.vector.tensor_copy(dst[:, st], sj[:])
                                else:
                                    nc.vector.scalar_tensor_tensor(
                                        out=dst[:, st], in0=dst[:, st],
                                        scalar=alpha[:], in1=sj[:],
                                        op0=ALU.mult, op1=ALU.add)

                            # transpose P blocks
                            pt = work.tile([128, KBLK, 128], AVDT, tag="pt")
                            for kb in range(KBLK):
                                ptp = ps_t.tile([128, 128], TDT, tag="ptp")
                                nc.tensor.transpose(
                                    ptp[:], p_sb[:, bass.ts(kb, 128)], ident[:])
                                nc.vector.tensor_copy(pt[:, kb], ptp[:])

                            # O_j = P^T-blocks @ V
                            op = ps_o.tile([128, D], F32, tag="op")
                            for dh in range(2):
                                for kb in range(KBLK):
                                    nc.tensor.matmul(
                                        op[:, bass.ds(dh * 512, 512)],
                                        pt[:, kb], vv[:, kb, bass.ds(dh * 512, 512)],
                                        start=(kb == 0), stop=(kb == KBLK - 1))

                            ot = slice(t * D, (t + 1) * D)
                            if NK == 1:
                                # single-chunk fragment: finalize straight
                                # from PSUM (no accumulator round-trip)
                                rec = small.tile([128, 1], F32, tag="rec")
                                nc.vector.reciprocal(rec[:], sj[:])
                                ofin = opool.tile([128, D], F32, tag="ofin")
                                nc.scalar.activation(
                                    ofin[:], op[:], AF.Copy, bias=0.0,
                                    scale=rec[:])
                                nc.sync.dma_start(o_e[qslot + t], ofin[:])
                                continue
                            if j == 0:
                                nc.scalar.copy(oacc[:, ot], op[:])
                            else:
                                nc.vector.scalar_tensor_tensor(
                                    out=oacc[:, ot], in0=oacc[:, ot],
                                    scalar=alpha[:], in1=op[:],
                                    op0=ALU.mult, op1=ALU.add)
                            if j == NK - 1:
                                # finalize this q-tile now: overlaps with the
                                # remaining tiles' compute instead of stacking
                                # at the fragment end
                                rec = small.tile([128, 1], F32, tag="rec")
                                nc.vector.reciprocal(rec[:], dst[:, st])
                                ofin = opool.tile([128, D], F32, tag="ofin")
                                nc.scalar.activation(
                                    ofin[:], oacc[:, ot], AF.Copy, bias=0.0,
                                    scale=rec[:])
                                nc.sync.dma_start(o_e[qslot + t], ofin[:])

                    qslot += NQ
                    chslot += NK

    nc.compile()
    return nc, TQ, CH


def _plan_multi(nqt, nkt):
    """Per-core specialized plans: returns frags[c] = [(b, q0, ln), ...] with
    near-perfectly balanced unit counts (unit = q-tile x k-chunk)."""
    total = sum(nqt[b] * nkt[b] for b in range(len(nqt)))

    def walk(order):
        frags = [[] for _ in range(NCORES)]
        c, done = 0, 0
        for b in order:
            q0 = 0
            while q0 < nqt[b]:
                quota_end = round((c + 1) * total / NCORES)
                room = quota_end - done
                if room < 0.75 * nkt[b] and c < NCORES - 1:
                    c += 1
                    continue
                ln = min(NQ_MAX, nqt[b] - q0, max(1, room // nkt[b]))
                frags[c].append((b, q0, ln))
                done += ln * nkt[b]
                q0 += ln
        return frags

    import random
    rng = random.Random(1234)
    base = sorted(range(len(nqt)), key=lambda b: -nkt[b])
    best, best_max = None, None
    for trial in range(300):
        order = list(base)
        if trial:
            rng.shuffle(order)
        fr = walk(order)
        mx = max(sum(ln * nkt[b] for (b, _, ln) in f) for f in fr)
        nfr = max(len(f) for f in fr)
        if best is None or (mx, nfr) < best_max:
            best, best_max = fr, (mx, nfr)
    frags = best

    # local rebalance: shave single q-tiles off the heaviest core
    def load(c):
        return sum(ln * nkt[b] for (b, _, ln) in frags[c])

    for _ in range(64):
        loads = [load(c) for c in range(NCORES)]
        hi = max(range(NCORES), key=lambda c: loads[c])
        lo = min(range(NCORES), key=lambda c: loads[c])
        gap = loads[hi] - loads[lo]
        cand = [i for i, (b, _, ln) in enumerate(frags[hi])
                if nkt[b] <= gap - nkt[b]]
        if not cand:
            break
        i = max(cand, key=lambda i: nkt[frags[hi][i][0]])
        b, q0, ln = frags[hi][i]
        if ln == 1:
            frags[hi].pop(i)
            frags[lo].append((b, q0, 1))
        else:
            frags[hi][i] = (b, q0, ln - 1)
            frags[lo].append((b, q0 + ln - 1, 1))
    return frags


# --------------------------------------------------------------------------
# cached PJRT executor (adapted from concourse.bass2jax.run_bass_via_pjrt)
# --------------------------------------------------------------------------

_EXEC_CACHE: dict = {}


def _get_exec(nc):
    import jax
    from concourse import bass2jax, mybir as _mb
    from jax.experimental.shard_map import shard_map
    from jax.sharding import Mesh, PartitionSpec

    key = id(nc)
    if key in _EXEC_CACHE:
        return _EXEC_CACHE[key]
    bass2jax.install_neuronx_cc_hook()
    assert not nc.dbg_addr or not nc.dbg_callbacks

    partition_name = nc.partition_id_tensor.name if nc.partition_id_tensor else None
    in_names, out_names, out_avals = [], [], []
    for alloc in nc.m.functions[0].allocations:
        if not isinstance(alloc, _mb.MemoryLocationSet):
            continue
        name = alloc.memorylocations[0].name
        if alloc.kind == "ExternalInput":
            if name != partition_name:
                in_names.append(name)
        elif alloc.kind == "ExternalOutput":
            shape = tuple(alloc.tensor_shape)
            dtype = _mb.dt.np(alloc.dtype)
            out_names.append(name)
            out_avals.append(jax.core.ShapedArray(shape, dtype))
    n_params = len(in_names)
    n_outs = len(out_avals)
    all_in_names = list(in_names) + list(out_names)
    if partition_name is not None:
        all_in_names.append(partition_name)
    donate = tuple(range(n_params, n_params + n_outs))

    def _body(*args):
        operands = list(args)
        if partition_name is not None:
            operands.append(bass2jax.partition_id_tensor())
        return tuple(bass2jax._bass_exec_p.bind(
            *operands,
            out_avals=tuple(out_avals),
            in_names=tuple(all_in_names),
            out_names=tuple(out_names),
            lowering_input_output_aliases=(),
            sim_require_finite=True,
            sim_require_nnan=True,
            nc=nc,
        ))

    devices = jax.devices()[:NCORES]
    mesh = Mesh(np.asarray(devices), ("core",))
    in_specs = (PartitionSpec("core"),) * (n_params + n_outs)
    out_specs = (PartitionSpec("core"),) * n_outs
    sharded = jax.jit(
        shard_map(_body, mesh=mesh, in_specs=in_specs, out_specs=out_specs,
                  check_rep=False),
        donate_argnums=donate, keep_unused=True)
    info = dict(sharded=sharded, in_names=in_names, out_names=out_names,
                out_avals=out_avals, mesh=mesh, n_params=n_params)
    _EXEC_CACHE[key] = info
    return info


def _concat_inputs(info, in_maps):
    return [np.concatenate([np.asarray(m[name]) for m in in_maps], axis=0)
            for name in info["in_names"]]


def _zero_outs(info):
    return [np.zeros((NCORES * a.shape[0], *a.shape[1:]), a.dtype)
            for a in info["out_avals"]]


def _execute(nc, in_maps):
    try:
        info = _get_exec(nc)
        concat_in = _concat_inputs(info, in_maps)
        out_arrs = info["sharded"](*concat_in, *_zero_outs(info))
        results = [
            {name: np.asarray(out_arrs[i]).reshape(
                NCORES, *info["out_avals"][i].shape)[c]
             for i, name in enumerate(info["out_names"])}
            for c in range(NCORES)
        ]
        if int(os.environ.get("ATTN_TIME", "0")):
            LAST_EXEC_NS[0] = _time_exec(
                nc, concat_in, int(os.environ.get("ATTN_TIME_ITERS", "3")))
        return results
    except Exception:
        # robust fallback: the canonical entry point (same underlying path,
        # uncached) — also covers non-axon native environments
        res = run_bass_kernel_spmd(nc, in_maps, core_ids=list(range(NCORES)))
        return res.results


def _get_exec_single(nc, device):
    """Single-device jit executor for one core's specialized program."""
    import jax
    from concourse import bass2jax, mybir as _mb

    key = id(nc)
    if key in _EXEC_CACHE:
        return _EXEC_CACHE[key]
    bass2jax.install_neuronx_cc_hook()
    partition_name = nc.partition_id_tensor.name if nc.partition_id_tensor else None
    in_names, out_names, out_avals = [], [], []
    for alloc in nc.m.functions[0].allocations:
        if not isinstance(alloc, _mb.MemoryLocationSet):
            continue
        name = alloc.memorylocations[0].name
        if alloc.kind == "ExternalInput":
            if name != partition_name:
                in_names.append(name)
        elif alloc.kind == "ExternalOutput":
            out_names.append(name)
            out_avals.append(jax.core.ShapedArray(
                tuple(alloc.tensor_shape), _mb.dt.np(alloc.dtype)))
    n_params = len(in_names)
    all_in_names = list(in_names) + list(out_names)
    if partition_name is not None:
        all_in_names.append(partition_name)
    donate = tuple(range(n_params, n_params + len(out_names)))

    def _body(*args):
        operands = list(args)
        if partition_name is not None:
            operands.append(bass2jax.partition_id_tensor())
        return tuple(bass2jax._bass_exec_p.bind(
            *operands,
            out_avals=tuple(out_avals),
            in_names=tuple(all_in_names),
            out_names=tuple(out_names),
            lowering_input_output_aliases=(),
            sim_require_finite=True,
            sim_require_nnan=True,
            nc=nc,
        ))

    fn = jax.jit(_body, donate_argnums=donate, keep_unused=True)
    info = dict(fn=fn, in_names=in_names, out_names=out_names,
                out_avals=out_avals, device=device)
    _EXEC_CACHE[key] = info
    return info


def _execute_multi(ncs, in_maps):
    """Run 8 per-core programs concurrently (async dispatch, then block)."""
    from concurrent.futures import ThreadPoolExecutor
    import jax
    from jax.sharding import SingleDeviceSharding
    devices = jax.devices()[:NCORES]
    infos = [_get_exec_single(ncs[c], devices[c]) for c in range(NCORES)]

    def precompile(c):
        info = infos[c]
        if info.get("compiled"):
            return
        sh = SingleDeviceSharding(devices[c])
        avals = [jax.ShapeDtypeStruct(np.asarray(in_maps[c][n]).shape,
                                      np.asarray(in_maps[c][n]).dtype, sharding=sh)
                 for n in info["in_names"]]
        avals += [jax.ShapeDtypeStruct(a.shape, a.dtype, sharding=sh)
                  for a in info["out_avals"]]
        info["fn"].lower(*avals).compile()
        info["compiled"] = True

    with ThreadPoolExecutor(NCORES) as ex:
        list(ex.map(precompile, range(NCORES)))

    futures = []
    for c in range(NCORES):
        info = infos[c]
        args = [jax.device_put(np.asarray(in_maps[c][n]), devices[c])
                for n in info["in_names"]]
        zeros = [jax.device_put(np.zeros(a.shape, a.dtype), devices[c])
                 for a in info["out_avals"]]
        futures.append((info, info["fn"](*args, *zeros)))
    results = []
    for info, outs in futures:
        results.append({name: np.asarray(outs[i])
                        for i, name in enumerate(info["out_names"])})
    return results


def _time_exec(nc, concat_in, iters=3):
    """Wall-clock the sharded execution with device-resident inputs."""
    import time
    import jax
    from jax.sharding import NamedSharding, PartitionSpec
    info = _get_exec(nc)
    sh = NamedSharding(info["mesh"], PartitionSpec("core"))
    dev_in = [jax.device_put(x, sh) for x in concat_in]
    for x in dev_in:
        x.block_until_ready()
    times = []
    for _ in range(iters):
        zeros = [jax.device_put(z, sh) for z in _zero_outs(info)]
        for z in zeros:
            z.block_until_ready()
        t0 = time.perf_counter()
        outs = info["sharded"](*dev_in, *zeros)
        for o in outs:
            o.block_until_ready()
        times.append(time.perf_counter() - t0)
    best = min(times)
    print(f"exec wall times: {[f'{t*1e3:.2f}ms' for t in times]}")
    return int(best * 1e9)


# --------------------------------------------------------------------------
# host entry
# --------------------------------------------------------------------------

def kernel(Q, K, V, Q_lengths, K_lengths):
    Q = np.ascontiguousarray(np.asarray(Q, dtype=np.float32))
    K = np.ascontiguousarray(np.asarray(K, dtype=np.float32))
    V = np.ascontiguousarray(np.asarray(V, dtype=np.float32))
    ql_i = np.asarray(Q_lengths).astype(np.int64)
    kl_i = np.asarray(K_lengths).astype(np.int64)

    nqt = [int(-(-min(max(q, 0), QT) // QTILE)) for q in ql_i]
    nkt = [int(-(-min(max(k, 1), KT) // KCH)) for k in kl_i]

    nprogs = int(os.environ.get("ATTN_NPROGS", "1"))
    multi_pass = QK_MODE == "bf16x3"
    if nprogs > 1:
        core_frags = _plan_multi(nqt, nkt)
        core_groups = [[(nkt[b], ln, 1) for (b, q0, ln) in fr]
                       for fr in core_frags]
        ncs, core_meta = [], []
        for c in range(NCORES):
            sig = (c, tuple(core_groups[c]), QK_MODE, "multi")
            if sig not in _PROG_CACHE:
                _PROG_CACHE[sig] = _build_program(core_groups[c], QK_MODE)
            ncs.append(_PROG_CACHE[sig][0])
            core_meta.append(_PROG_CACHE[sig][1:])
    else:
        groups, assign = _plan(nqt, nkt, _DMA_R.get(QK_MODE, 2.5))
        sig = (tuple(groups), QK_MODE)
        if sig not in _PROG_CACHE:
            _PROG_CACHE[sig] = _build_program(groups, QK_MODE)
        nc, TQ, CH = _PROG_CACHE[sig]

    # precompute split/rounded operands
    if QK_MODE == "fp16":
        Qh, Kh = Q.astype(np.float16), K.astype(np.float16)
        Vr = V.astype(np.float16)
        qdt = avdt = np.float16
        maskval = -60000.0  # fp16-exact; exp(S+mask-m) underflows to 0 exactly
    elif QK_MODE == "tf32":
        Qh, Kh = _tf32(Q), _tf32(K)
        Vr = _tf32(V)
        qdt = avdt = np.float32
        maskval = MASK_NEG
    else:
        Qh = Q.astype(ml_dtypes.bfloat16)
        Ql = (Q - Qh.astype(np.float32)).astype(ml_dtypes.bfloat16)
        Kh = K.astype(ml_dtypes.bfloat16)
        Kl = (K - Kh.astype(np.float32)).astype(ml_dtypes.bfloat16)
        Vr = _tf32(V)
        qdt, avdt = ml_dtypes.bfloat16, np.float32
        maskval = MASK_NEG

    def pack_core(groups_c, run_for, TQ_c, CH_c):
        qh_a = np.zeros((TQ_c, DCH, 128, QTILE), dtype=qdt)
        kh_a = np.zeros((CH_c, DCH, 128, KCH), dtype=qdt)
        if multi_pass:
            ql_a = np.zeros((TQ_c, DCH, 128, QTILE), dtype=ml_dtypes.bfloat16)
            kl_a = np.zeros((CH_c, DCH, 128, KCH), dtype=ml_dtypes.bfloat16)
        v_a = np.zeros((CH_c, KBLK, 128, D), dtype=avdt)
        mk_a = np.full((CH_c, 128, KCH), maskval, dtype=qdt)
        qslot = chslot = 0
        for gi, (NK, NQ, F) in enumerate(groups_c):
            for f in range(F):
                run = run_for(gi, f)
                if run is not None:
                    b, q0, ln = run
                    for t in range(ln):
                        qt = q0 + t
                        blk = Qh[b, qt * QTILE:(qt + 1) * QTILE, :].T  # [D,128]
                        qh_a[qslot + t] = blk.reshape(DCH, 128, QTILE)
                        if multi_pass:
                            blk = Ql[b, qt * QTILE:(qt + 1) * QTILE, :].T
                            ql_a[qslot + t] = blk.reshape(DCH, 128, QTILE)
                    klen = int(min(max(kl_i[b], 1), KT))
                    for j in range(min(nkt[b], NK)):
                        ksl = slice(j * KCH, (j + 1) * KCH)
                        kh_a[chslot + j] = Kh[b, ksl, :].T.reshape(DCH, 128, KCH)
                        if multi_pass:
                            kl_a[chslot + j] = Kl[b, ksl, :].T.reshape(
                                DCH, 128, KCH)
                        v_a[chslot + j] = Vr[b, ksl, :].reshape(KBLK, 128, D)
                        kk = np.arange(j * KCH, (j + 1) * KCH)
                        mk_a[chslot + j] = np.where(kk < klen, 0.0,
                                                    maskval).astype(qdt)
                qslot += NQ
                chslot += NK
        m = {"qh": qh_a, "kh": kh_a, "v": v_a, "mask": mk_a,
             "ident": np.eye(128, dtype=np.float16 if QK_MODE == "fp16"
                             else np.float32)}
        if multi_pass:
            m["ql"] = ql_a
            m["kl"] = kl_a
        return m

    def unpack_core(groups_c, run_for, o_a, out, done):
        qslot = 0
        for gi, (NK, NQ, F) in enumerate(groups_c):
            for f in range(F):
                run = run_for(gi, f)
                if run is not None:
                    b, q0, ln = run
                    for t in range(ln):
                        out[b, (q0 + t) * QTILE:(q0 + t + 1) * QTILE, :] = \
                            o_a[qslot + t]
                        done[b, q0 + t] = True
                qslot += NQ

    out = np.empty((B, QT, D), dtype=np.float32)
    v_mean = V.mean(axis=1, dtype=np.float64).astype(np.float32)  # [B, D]
    done = np.zeros((B, QT // QTILE), dtype=bool)

    if nprogs > 1:
        in_maps = []
        for c in range(NCORES):
            TQ_c, CH_c = core_meta[c]
            run_for = lambda gi, f, fr=core_frags[c]: fr[gi]
            in_maps.append(pack_core(core_groups[c], run_for, TQ_c, CH_c))
        results = _execute_multi(ncs, in_maps)
        for c in range(NCORES):
            run_for = lambda gi, f, fr=core_frags[c]: fr[gi]
            unpack_core(core_groups[c], run_for, results[c]["o"], out, done)
    else:
        in_maps = [
            pack_core(groups,
                      lambda gi, f, cc=c: assign.get((cc, gi, f)), TQ, CH)
            for c in range(NCORES)
        ]
        results = _execute(nc, in_maps)
        for c in range(NCORES):
            unpack_core(groups,
                        lambda gi, f, cc=c: assign.get((cc, gi, f)),
                        results[c]["o"], out, done)

    # rows q >= Q_len: reference yields uniform average over ALL of V
    for b in range(B):
        qlen = int(min(max(ql_i[b], 0), QT))
        out[b, qlen:, :] = v_mean[b]
        assert done[b, :nqt[b]].all()
    return out
